# revision 1
# baseline (speedup 1.0000x reference)
import sys
if '/opt/trn_rl_repo' not in sys.path:
    sys.path.insert(0, '/opt/trn_rl_repo')
import numpy as np
import concourse.bacc as bacc
import concourse.mybir as mybir
import concourse.tile as tile
from concourse.bass_utils import run_bass_kernel_spmd

dt = mybir.dt
AF = mybir.ActivationFunctionType
OP = mybir.AluOpType
BF16 = dt.np(dt.bfloat16)

# problem shapes (hardcoded per spec)
T = 2048
D = 2048
H = 16
HD = 128
FFN = 8192
CORES = 8
HPC = H // CORES          # heads per core = 2
E = HPC * HD              # per-core attention feature width = 256
FSH = FFN // CORES        # ffn rows per core = 1024
CH = 512                  # t-chunk for everything
NT = T // CH
ND = D // 128
NF = FSH // 128
EPS = float(np.finfo(np.float32).eps)

_CACHE = {}
LAST_RESULT = None


def _attn_band(docs, ch):
    """Per t-chunk (width ch) list of (st, full, mask[128,ch] or None)."""
    docs = np.asarray(docs).astype(np.int64)
    is_sorted = bool(np.all(np.diff(docs) >= 0))
    out = []
    for tc in range(T // ch):
        t0, t1 = tc * ch, (tc + 1) * ch
        if is_sorted:
            s_lo = int(np.searchsorted(docs, docs[t0], side='left'))
            st_lo = s_lo // 128
        else:
            st_lo = 0
        st_hi = (t1 - 1) // 128
        tiles = []
        for st in range(st_lo, st_hi + 1):
            s0, s1 = st * 128, (st + 1) * 128
            m = (np.arange(t0, t1)[None, :] >= np.arange(s0, s1)[:, None]) \
                & (docs[None, t0:t1] == docs[s0:s1, None])
            if not m.any():
                continue
            full = bool(m.all())
            tiles.append((st, full, None if full else m.astype(np.float32)))
        out.append(tiles)
    return out


def _build(band, scale, key_offset):
    """SPMD Bass program (identical on all cores).

    o-proj is a contraction-split partial over the core's own two heads
    with x/8 folded in; one bf16 AllReduce per 512-token chunk yields the
    full residual x1 on every core. The MLP runs on raw x1 with the
    rms-norm factor folded into the proj output as rcp^2.
    """
    nc = bacc.Bacc("TRN2", target_bir_lowering=False, debug=False,
                   num_devices=CORES)
    f32, f32r, bf = dt.float32, dt.float32r, dt.bfloat16

    mask_idx = {}
    for tcc, tiles in enumerate(band):
        for (st, full, m) in tiles:
            if not full:
                mask_idx[(tcc, st)] = len(mask_idx)
    n_masks = max(len(mask_idx), 1)

    # ---- DRAM I/O ----
    xT_d   = nc.dram_tensor("xT",   [D, T], bf, kind="ExternalInput")
    wqkv_d = nc.dram_tensor("wqkvT", [D, 3 * E], bf, kind="ExternalInput")
    wg_d   = nc.dram_tensor("wgT",  [D, 128], bf, kind="ExternalInput")
    wo_d   = nc.dram_tensor("woT",  [E, D], bf, kind="ExternalInput")
    wfc_d  = nc.dram_tensor("wfcT", [D, FSH], bf, kind="ExternalInput")
    wpr_d  = nc.dram_tensor("wpr",  [FSH, D], bf, kind="ExternalInput")
    ve_d   = nc.dram_tensor("veS",  [T, E], bf, kind="ExternalInput")
    ropeA_d = nc.dram_tensor("ropeA", [HD, T], bf, kind="ExternalInput")
    ropeB_d = nc.dram_tensor("ropeB", [HD, T], bf, kind="ExternalInput")
    onesb_d = nc.dram_tensor("onesb", [128, 1], bf, kind="ExternalInput")
    eps_d  = nc.dram_tensor("epsb", [128, 1], f32, kind="ExternalInput")
    mask_d = nc.dram_tensor("masks", [n_masks, 128, CH], bf,
                            kind="ExternalInput")

    mlp_d  = nc.dram_tensor("mlpT", [D, T], bf, kind="ExternalOutput")
    x1_d   = nc.dram_tensor("x1T", [D, T], bf, kind="ExternalOutput")

    # qkv block order: q0,k0,q1,k1,v0,v1 (q/k adjacent per head for the
    # grouped rms squares)
    v_of = 2 * HPC
    XSPLIT = 4
    NCH = CH // 128
    SG = 4               # n-tiles per grouped square

    with tile.TileContext(nc) as tc_:
        with tc_.tile_pool(name="const", bufs=1) as const, \
             tc_.tile_pool(name="dram", bufs=1, space="DRAM") as dram, \
             tc_.tile_pool(name="sW", bufs=1) as sW, \
             tc_.tile_pool(name="pAB", bufs=1) as pAB:
            onesb = const.tile([128, 1], bf)
            nc.sync.dma_start(onesb[:], onesb_d.ap())
            epsb = const.tile([128, 1], f32)
            nc.sync.dma_start(epsb[:], eps_d.ap())

            wo = sW.tile([128, HPC, D], bf)   # DMA issued after chunk 0

            cc_in = [dram.tile([D, CH], bf, name=f"cci{t}")
                     for t in range(NT)]
            cc_out = [dram.tile([D, CH], bf, addr_space="Shared",
                                name=f"cco{t}") for t in range(NT)]

            # persistent across phases A+B: attention operands
            gates = pAB.tile([128, T], bf)
            qrs = [pAB.tile([128, T], bf, name=f"qr{h}") for h in range(HPC)]
            kfs = [pAB.tile([128, T], bf, name=f"kf{h}") for h in range(HPC)]
            vuses = [pAB.tile([128, T // 128, HD], bf, name=f"vu{h}")
                     for h in range(HPC)]

            # ===== Phase A: qkv + gates + per-chunk attention prep =====
            with tc_.tile_pool(name="stA", bufs=1) as sA, \
                 tc_.tile_pool(name="xtp", bufs=2) as xtp, \
                 tc_.tile_pool(name="stA2", bufs=2) as sA2, \
                 tc_.tile_pool(name="psA", bufs=4, space="PSUM") as psA, \
                 tc_.tile_pool(name="psRow", bufs=2, space="PSUM") as psRow:
                wq = sA.tile([128, ND, 3 * E], bf)
                wqr = wqkv_d.ap().rearrange("(n p) e -> p n e", p=128)
                for ws in range(4):
                    nc.sync.dma_start(wq[:, ws * 4:(ws + 1) * 4],
                                      wqr[:, ws * 4:(ws + 1) * 4])
                wg = sA.tile([128, ND, 128], bf)
                nc.sync.dma_start(
                    wg[:], wg_d.ap().rearrange("(n p) e -> p n e", p=128))
                ropeA = sA.tile([HD, T], bf)
                ropeB = sA.tile([HD, T], bf)
                qkvT = sA.tile([128, 2 * HPC, T], bf)        # q0,k0,q1,k1
                krs = [sA.tile([128, T], bf, name=f"kr{h}") for h in range(HPC)]
                ve_nat = sA.tile([128, T // 128, E], bf)
                gcolT = sA.tile([128, T // 128, 128], bf)
                xTr = xT_d.ap().rearrange("(n p) t -> p n t", p=128)
                for t in range(NT):
                    ts = slice(t * CH, (t + 1) * CH)
                    xt = xtp.tile([128, ND, CH], bf, tag="xt")
                    nsub = ND // XSPLIT
                    for xs in range(XSPLIT):
                        nc.sync.dma_start(
                            xt[:, xs * nsub:(xs + 1) * nsub],
                            xTr[:, xs * nsub:(xs + 1) * nsub, ts])
                    if t == 0:
                        nc.sync.dma_start(ropeA[:], ropeA_d.ap())
                        nc.sync.dma_start(ropeB[:], ropeB_d.ap())
                        nc.sync.dma_start(
                            ve_nat[:],
                            ve_d.ap().rearrange("(n p) e -> p n e", p=128))
                        nc.sync.dma_start(
                            wo[:],
                            wo_d.ap().rearrange("(et p) d -> p et d", p=128))
                    # rms rowsum: grouped squares (bf16) + skinny MMs
                    pr = psRow.tile([1, CH], f32, tag="row")
                    for g in range(ND // SG):
                        sq = sA2.tile([128, SG, CH], bf, tag="sq")
                        nc.scalar.activation(sq[:], xt[:, g * SG:(g + 1) * SG],
                                             AF.Square)
                        for j in range(SG):
                            n = g * SG + j
                            nc.tensor.matmul(pr[:], onesb[:], sq[:, j],
                                             start=(n == 0),
                                             stop=(n == ND - 1))
                    sd = sA2.tile([1, CH], f32, tag="sd")
                    nc.scalar.activation(sd[:], pr[:], AF.Sqrt,
                                         bias=epsb[0:1, :], scale=1.0 / D)
                    rcp = sA2.tile([1, CH], f32, tag="rcp")
                    nc.vector.reciprocal_approx_fast(rcp[:], sd[:])
                    rep = sA2.tile([128, CH], f32, tag="rep")
                    nc.gpsimd.partition_broadcast(rep[:], rcp[:])
                    # q, k projections -> qkvT
                    for m in range(2 * HPC):
                        pq = psA.tile([128, CH], f32, tag="mm")
                        for n in range(ND):
                            nc.tensor.matmul(
                                pq[:], wq[:, n, m * 128:(m + 1) * 128],
                                xt[:, n], start=(n == 0), stop=(n == ND - 1))
                        nc.vector.tensor_tensor(qkvT[:, m, ts], pq[:],
                                                rep[:], OP.mult)
                    # gates (before v so the gcol transpose lands early)
                    pg = psA.tile([128, CH], f32, tag="mm")
                    for n in range(ND):
                        nc.tensor.matmul(pg[:], wg[:, n], xt[:, n],
                                         start=(n == 0), stop=(n == ND - 1))
                    gm = sA2.tile([128, CH], f32, tag="gm")
                    nc.vector.tensor_tensor(gm[:], pg[:], rep[:], OP.mult)
                    nc.scalar.activation(gates[:, ts], gm[:], AF.Sigmoid)
                    nc.sync.dma_start_transpose(
                        gcolT[:, t * NCH:(t + 1) * NCH], gates[:, ts])
                    # v projections -> transient, xbar-transposed below
                    vtmps = []
                    for h in range(HPC):
                        m = v_of + h
                        pq = psA.tile([128, CH], f32, tag="mm")
                        for n in range(ND):
                            nc.tensor.matmul(
                                pq[:], wq[:, n, m * 128:(m + 1) * 128],
                                xt[:, n], start=(n == 0), stop=(n == ND - 1))
                        vtmp = sA2.tile([128, CH], bf, tag=f"vtmp{h}")
                        nc.vector.tensor_tensor(vtmp[:], pq[:], rep[:],
                                                OP.mult)
                        vtmps.append(vtmp)

                    # ---- per-chunk attention prep (overlaps next qkv) ----
                    hh = HD // 2
                    for h in range(HPC):
                        q = qkvT[:, 2 * h]
                        k = qkvT[:, 2 * h + 1]
                        qr, kr, kf = qrs[h], krs[h], kfs[h]
                        # grouped rms rowsums for q,k (adjacent blocks)
                        sq2 = sA2.tile([128, 2, CH], bf, tag="sq2")
                        nc.scalar.activation(sq2[:], qkvT[:, 2 * h:2 * h + 2, ts],
                                             AF.Square)
                        reps = []
                        for j in range(2):
                            pr2 = psRow.tile([1, CH], f32, tag="row")
                            nc.tensor.matmul(pr2[:], onesb[:], sq2[:, j],
                                             start=True, stop=True)
                            sd2 = sA2.tile([1, CH], f32, tag=f"sd2{j}")
                            nc.scalar.activation(sd2[:], pr2[:], AF.Sqrt,
                                                 bias=epsb[0:1, :],
                                                 scale=1.0 / HD)
                            rcp2 = sA2.tile([1, CH], f32, tag=f"rcp2{j}")
                            nc.vector.reciprocal_approx_fast(rcp2[:], sd2[:])
                            rep2 = sA2.tile([128, CH], f32, tag=f"rep2{j}")
                            nc.gpsimd.partition_broadcast(rep2[:], rcp2[:])
                            reps.append(rep2)
                        # rotary on raw q/k (rms-norm commutes with rope)
                        tmp = sA2.tile([128, CH], bf, tag="tmp")
                        for (src_, dst, rep2) in ((q, qr, reps[0]),
                                                  (k, kr, reps[1])):
                            x1h, x2h = src_[0:hh, ts], src_[hh:HD, ts]
                            cosA, sinA = ropeA[0:hh, ts], ropeA[hh:HD, ts]
                            sinB, cosB = ropeB[0:hh, ts], ropeB[hh:HD, ts]
                            nc.vector.tensor_tensor(tmp[0:hh, :], x2h, sinA,
                                                    OP.mult)
                            nc.vector.tensor_tensor(dst[0:hh, ts], x1h,
                                                    cosA, OP.mult)
                            nc.vector.tensor_tensor(dst[0:hh, ts],
                                                    dst[0:hh, ts],
                                                    tmp[0:hh, :], OP.add)
                            nc.vector.tensor_tensor(tmp[hh:HD, :], x1h,
                                                    sinB, OP.mult)
                            nc.vector.tensor_tensor(dst[hh:HD, ts], x2h,
                                                    cosB, OP.mult)
                            nc.vector.tensor_tensor(dst[hh:HD, ts],
                                                    dst[hh:HD, ts],
                                                    tmp[hh:HD, :],
                                                    OP.subtract)
                            nc.vector.tensor_tensor(dst[:, ts], dst[:, ts],
                                                    rep2[:], OP.mult)
                        # key_offset shift on gpsimd (quarters 1,3 read t-1)
                        a_, b_, c3 = HD // 4, HD // 2, 3 * HD // 4
                        t0 = t * CH
                        nc.gpsimd.tensor_copy(kf[0:a_, ts], kr[0:a_, ts])
                        nc.gpsimd.tensor_copy(kf[b_:c3, ts], kr[b_:c3, ts])
                        if key_offset:
                            if t == 0:
                                nc.gpsimd.tensor_copy(kf[a_:b_, 0:1],
                                                      kr[a_:b_, 0:1])
                                nc.gpsimd.tensor_copy(kf[c3:HD, 0:1],
                                                      kr[c3:HD, 0:1])
                                nc.gpsimd.tensor_copy(kf[a_:b_, 1:CH],
                                                      kr[a_:b_, 0:CH - 1])
                                nc.gpsimd.tensor_copy(kf[c3:HD, 1:CH],
                                                      kr[c3:HD, 0:CH - 1])
                            else:
                                nc.gpsimd.tensor_copy(
                                    kf[a_:b_, t0:t0 + CH],
                                    kr[a_:b_, t0 - 1:t0 + CH - 1])
                                nc.gpsimd.tensor_copy(
                                    kf[c3:HD, t0:t0 + CH],
                                    kr[c3:HD, t0 - 1:t0 + CH - 1])
                        else:
                            nc.gpsimd.tensor_copy(kf[a_:b_, ts],
                                                  kr[a_:b_, ts])
                            nc.gpsimd.tensor_copy(kf[c3:HD, ts],
                                                  kr[c3:HD, ts])
                        # v: xbar transpose, then add gated ve
                        vtT = sA2.tile([128, NCH, HD], bf, tag="vtT")
                        nc.sync.dma_start_transpose(vtT[:], vtmps[h][:])
                        for j in range(NCH):
                            st = t * NCH + j
                            nc.vector.scalar_tensor_tensor(
                                vuses[h][:, st],
                                ve_nat[:, st, h * HD:(h + 1) * HD],
                                gcolT[:, st, 32 * h:32 * h + 1],
                                vtT[:, j], OP.mult, OP.add)

            # ===== Phase B: attention + o-partials + AllReduce =====
            with tc_.tile_pool(name="sW2", bufs=1) as sW2:
                wfc = sW2.tile([128, ND, FSH], bf)
                wfcr = wfc_d.ap().rearrange("(n p) f -> p n f", p=128)
                for ws in range(8):
                    nc.sync.dma_start(wfc[:, ws * 2:(ws + 1) * 2],
                                      wfcr[:, ws * 2:(ws + 1) * 2])
                wpr = sW2.tile([128, NF, D], bf)
                wprr = wpr_d.ap().rearrange("(n p) d -> p n d", p=128)
                for ws in range(8):
                    nc.sync.dma_start(wpr[:, ws:ws + 1], wprr[:, ws:ws + 1])

                with tc_.tile_pool(name="stB2", bufs=2) as sB2, \
                     tc_.tile_pool(name="mskp", bufs=2) as mskp, \
                     tc_.tile_pool(name="ptp", bufs=2) as ptp, \
                     tc_.tile_pool(name="yp", bufs=2) as yp, \
                     tc_.tile_pool(name="xtp2", bufs=1) as xtp2, \
                     tc_.tile_pool(name="osbp", bufs=2) as osbp, \
                     tc_.tile_pool(name="psS", bufs=1, space="PSUM") as psS, \
                     tc_.tile_pool(name="psY", bufs=2, space="PSUM") as psY, \
                     tc_.tile_pool(name="psD", bufs=1, space="PSUM") as psD, \
                     tc_.tile_pool(name="psO", bufs=2, space="PSUM") as psO:
                    for t4 in range(NT):
                        tiles = band[t4]
                        nti = len(tiles)
                        ts = slice(t4 * CH, (t4 + 1) * CH)
                        yTt = yp.tile([128, HPC, CH], bf, tag="yT")
                        mks = {}
                        for i, (st, full, _m) in enumerate(tiles):
                            if not full:
                                mk = mskp.tile([128, CH], bf, tag=f"mk{i}")
                                nc.sync.dma_start(
                                    mk[:], mask_d.ap()[mask_idx[(t4, st)]])
                                mks[i] = mk
                        for h in range(HPC):
                            qr, kf, vuse = qrs[h], kfs[h], vuses[h]
                            py = psY.tile([128, CH], f32, tag="py")
                            pdn = psD.tile([1, CH], f32, tag="pd")
                            # software-pipelined: score(i) runs 2 ahead of
                            # pv(i)/pden(i) so PE stays dense on 3 ps tiles
                            pts = {}

                            def score(i):
                                st = tiles[i][0]
                                pst = psS.tile([128, CH], f32,
                                               tag=f"s{i % 3}")
                                nc.tensor.matmul(
                                    pst[:], kf[:, st * 128:(st + 1) * 128],
                                    qr[:, ts], start=True, stop=True)
                                pT = ptp.tile([128, CH], bf, tag=f"pT{i % 3}")
                                nc.scalar.activation(pT[:], pst[:], AF.Exp,
                                                     scale=scale)
                                if not tiles[i][1]:
                                    nc.vector.tensor_tensor(
                                        pT[:], pT[:], mks[i][:], OP.mult)
                                pts[i] = pT

                            def pvpd(i):
                                st = tiles[i][0]
                                nc.tensor.matmul(
                                    py[:], vuse[:, st], pts[i][:],
                                    start=(i == 0), stop=(i == nti - 1))
                                nc.tensor.matmul(
                                    pdn[:], onesb[:], pts[i][:],
                                    start=(i == 0), stop=(i == nti - 1))

                            for i in range(nti):
                                score(i)
                                if i >= 2:
                                    pvpd(i - 2)
                            for i in range(max(0, nti - 2), nti):
                                pvpd(i)
                            rec = sB2.tile([1, CH], f32, tag="rec")
                            nc.vector.reciprocal_approx_fast(rec[:], pdn[:])
                            grow = sB2.tile([1, CH], f32, tag="grow")
                            nc.vector.tensor_copy(
                                grow[:], gates[64 + 32 * h:65 + 32 * h, ts])
                            comb = sB2.tile([1, CH], f32, tag="comb")
                            nc.vector.tensor_tensor(comb[:], rec[:], grow[:],
                                                    OP.mult)
                            repy = sB2.tile([128, CH], f32, tag=f"repy{h}")
                            nc.gpsimd.partition_broadcast(repy[:], comb[:])
                            nc.vector.tensor_tensor(yTt[:, h], py[:],
                                                    repy[:], OP.mult)
                        # o-proj partial + x/8, AllReduce -> full x1
                        xt2 = xtp2.tile([128, ND, CH], bf, tag="xt2")
                        nc.sync.dma_start(
                            xt2[:], xT_d.ap().rearrange(
                                "(n p) t -> p n t", p=128)[:, :, ts])
                        ccr = cc_in[t4][:].rearrange("(n p) c -> p n c", p=128)
                        for m in range(ND):
                            po = psO.tile([128, CH], f32, tag="po")
                            nc.tensor.matmul(
                                po[:], wo[:, 0, m * 128:(m + 1) * 128],
                                yTt[:, 0], start=True, stop=False)
                            nc.tensor.matmul(
                                po[:], wo[:, 1, m * 128:(m + 1) * 128],
                                yTt[:, 1], start=False, stop=True)
                            osb = osbp.tile([128, CH], bf, tag=f"osb{m % 4}")
                            nc.vector.scalar_tensor_tensor(
                                osb[:], xt2[:, m], 1.0 / CORES, po[:],
                                OP.mult, OP.add)
                            nc.sync.dma_start(ccr[:, m], osb[:])
                        nc.gpsimd.collective_compute(
                            "AllReduce", OP.add,
                            replica_groups=[list(range(CORES))],
                            ins=[cc_in[t4][:].opt()],
                            outs=[cc_out[t4][:].opt()])

                # ===== Phase C: MLP on full x1 =====
                with tc_.tile_pool(name="sCD", bufs=2) as sCD, \
                     tc_.tile_pool(name="x1p", bufs=2) as x1p, \
                     tc_.tile_pool(name="hp", bufs=2) as hp, \
                     tc_.tile_pool(name="otp", bufs=1) as otp, \
                     tc_.tile_pool(name="psH", bufs=2, space="PSUM") as psH, \
                     tc_.tile_pool(name="psM", bufs=2, space="PSUM") as psM, \
                     tc_.tile_pool(name="psR3", bufs=2, space="PSUM") as psR3:
                    mlpr = mlp_d.ap().rearrange("(n p) t -> p n t", p=128)
                    x1r = x1_d.ap().rearrange("(n p) t -> p n t", p=128)
                    for t in range(NT):
                        ts = slice(t * CH, (t + 1) * CH)
                        x1t = x1p.tile([128, ND, CH], bf, tag="x1t")
                        nc.sync.dma_start(
                            x1t[:], cc_out[t][:].rearrange(
                                "(n p) c -> p n c", p=128))
                        nc.sync.dma_start(x1r[:, :, ts], x1t[:])
                        pr3 = psR3.tile([1, CH], f32, tag="r3")
                        for g in range(ND // SG):
                            sq3 = sCD.tile([128, SG, CH], bf, tag="sq3")
                            nc.scalar.activation(sq3[:],
                                                 x1t[:, g * SG:(g + 1) * SG],
                                                 AF.Square)
                            for j in range(SG):
                                n = g * SG + j
                                nc.tensor.matmul(pr3[:], onesb[:], sq3[:, j],
                                                 start=(n == 0),
                                                 stop=(n == ND - 1))
                        sd3 = sCD.tile([1, CH], f32, tag="sd3")
                        nc.scalar.activation(sd3[:], pr3[:], AF.Sqrt,
                                             bias=epsb[0:1, :], scale=1.0 / D)
                        rcp3 = sCD.tile([1, CH], f32, tag="rcp3")
                        nc.vector.reciprocal_approx_fast(rcp3[:], sd3[:])
                        rsq = sCD.tile([1, CH], f32, tag="rsq")
                        nc.vector.tensor_tensor(rsq[:], rcp3[:], rcp3[:],
                                                OP.mult)
                        rep3 = sCD.tile([128, CH], f32, tag="rep3")
                        nc.gpsimd.partition_broadcast(rep3[:], rsq[:])
                        hT = hp.tile([128, NF, CH], bf, tag="hT")
                        for mf in range(NF):
                            ph = psH.tile([128, CH], f32, tag="h")
                            for n in range(ND):
                                nc.tensor.matmul(
                                    ph[:], wfc[:, n, mf * 128:(mf + 1) * 128],
                                    x1t[:, n], start=(n == 0),
                                    stop=(n == ND - 1))
                            hr = sCD.tile([128, CH], f32, tag="hr")
                            nc.scalar.activation(hr[:], ph[:], AF.Relu)
                            nc.gpsimd.tensor_tensor(hT[:, mf], hr[:], hr[:],
                                                    OP.mult)
                        otb = otp.tile([128, ND, CH], bf, tag="otb")
                        for md in range(ND):
                            pm = psM.tile([128, CH], f32, tag="m")
                            for mf in range(NF):
                                nc.tensor.matmul(
                                    pm[:], wpr[:, mf, md * 128:(md + 1) * 128],
                                    hT[:, mf], start=(mf == 0),
                                    stop=(mf == NF - 1))
                            nc.vector.tensor_tensor(otb[:, md], pm[:],
                                                    rep3[:], OP.mult)
                        nc.sync.dma_start(mlpr[:, :, ts], otb[:])

    nc.compile()
    return nc


def kernel(x, ve, qkvo_w, attn_gate_w, ve_gate_w, c_fc, c_proj,
           sa_lambdas, cos, sin, attn_scale, docs, key_offset):
    global LAST_RESULT
    x = np.asarray(x, np.float32)
    ve = np.asarray(ve, np.float32)
    qkvo_w = np.asarray(qkvo_w, np.float32)
    attn_gate_w = np.asarray(attn_gate_w, np.float32)
    ve_gate_w = np.asarray(ve_gate_w, np.float32)
    c_fc = np.asarray(c_fc, np.float32)
    c_proj = np.asarray(c_proj, np.float32)
    sa = np.asarray(sa_lambdas, np.float32)
    docs = np.asarray(docs)
    ko = int(np.asarray(key_offset))
    scale = float(np.asarray(attn_scale).reshape(-1)[0])

    band = _attn_band(docs, CH)
    key = (docs.tobytes(), scale, ko)
    if key not in _CACHE:
        _CACHE[key] = _build(band, scale, ko)
    nc = _CACHE[key]

    xT = np.ascontiguousarray(x[0].T)                       # [D, T] f32
    w_qkv = sa[0] * qkvo_w[:3 * D]                          # [3D, D]
    w_o = sa[1] * qkvo_w[3 * D:]                            # [D, D]
    cosT = np.asarray(cos, np.float32).T
    sinT = np.asarray(sin, np.float32).T
    ropeA = np.ascontiguousarray(np.concatenate([cosT, sinT], 0)).astype(BF16)
    ropeB = np.ascontiguousarray(np.concatenate([sinT, cosT], 0)).astype(BF16)
    onesb = np.ones((128, 1)).astype(BF16)
    epsb_np = np.full((128, 1), EPS, np.float32)
    xT_bf = xT.astype(BF16)

    mask_list = []
    for tcc, tiles in enumerate(band):
        for (st, full, m) in tiles:
            if not full:
                mask_list.append(m)
    if not mask_list:
        mask_list = [np.zeros((128, CH), np.float32)]
    masks = np.ascontiguousarray(np.stack(mask_list, 0)).astype(BF16)

    in_maps = []
    for c in range(CORES):
        g0 = c * HPC
        rows = []
        for h in range(HPC):                    # q0,k0,q1,k1 then v0,v1
            g = g0 + h
            rows.append(w_qkv[g * HD:(g + 1) * HD])                  # q_h
            rows.append(w_qkv[D + g * HD:D + (g + 1) * HD])          # k_h
        for h in range(HPC):
            g = g0 + h
            rows.append(w_qkv[2 * D + g * HD:2 * D + (g + 1) * HD])  # v_h
        wqkvT = np.ascontiguousarray(np.concatenate(rows, 0).T).astype(BF16)
        woT = np.ascontiguousarray(
            w_o[:, c * E:(c + 1) * E].T).astype(BF16)       # [E, D]
        wfcT = np.ascontiguousarray(c_fc[c * FSH:(c + 1) * FSH].T).astype(BF16)
        wpr = np.ascontiguousarray(c_proj[c * FSH:(c + 1) * FSH]).astype(BF16)
        wgT = np.zeros((D, 128), np.float32)
        wgT[:, 0] = ve_gate_w[g0]
        wgT[:, 32] = ve_gate_w[g0 + 1]
        wgT[:, 64] = attn_gate_w[g0]
        wgT[:, 96] = attn_gate_w[g0 + 1]
        wgT = wgT.astype(BF16)
        veS = np.ascontiguousarray(
            ve[0][:, g0 * HD:(g0 + HPC) * HD]).astype(BF16)         # [T, E]
        in_maps.append(dict(
            xT=xT_bf, wqkvT=wqkvT, woT=woT, wfcT=wfcT, wpr=wpr,
            wgT=wgT, veS=veS, ropeA=ropeA, ropeB=ropeB,
            onesb=onesb, epsb=epsb_np, masks=masks))

    res = run_bass_kernel_spmd(nc, in_maps, core_ids=list(range(CORES)))
    LAST_RESULT = res

    outT = res.results[0]["mlpT"].astype(np.float64)
    for c in range(1, CORES):
        outT += res.results[c]["mlpT"]
    outT += res.results[0]["x1T"].astype(np.float64)
    return np.ascontiguousarray(outT.T).astype(np.float32).reshape(1, T, D)

